# revision 8
# baseline (speedup 1.0000x reference)
"""MoE routing + task-head kernel for 8 Trainium2 NeuronCores.

Strategy (self-contained, shapes hardcoded from the problem):
  - Host (numpy): router softmax/top-2, capacity positions, gates, aux loss,
    and a balanced token->core assignment (2048 tokens/core) that equalizes
    per-core per-task counts and per-(core,expert) slot counts.  The host
    pre-gathers each core's tokens into an expert-grouped, feature-major
    dispatch buffer xbufT [D, E*P_E].
  - Device (8-way SPMD, one NEFF): per core
      phase 1: per-expert FFN  hT = relu(W1^T-tiles @ xT + b1),
               eo = hT-tiles @ W2 + b2  -> DRAM [E*P_E, D] (row-major)
      phase 2: combine  y = g0*eo[slot0] + g1*eo[slot1] via indirect row
               gathers, then PE-transpose y -> yT [D, TT]
      phase 3: task heads  o_i = lrelu(yT-block @ w0_i + b0_i) @ w1_i + b1_i
  - Host: scatter per-core head outputs back to global row order, return
    (task_indices, task_outs, aux_loss) exactly like the reference.

All matmuls run as float32r (full-rate PE mode, fp32 storage).
"""

import os

import numpy as np

N = 16384
D = 768
E = 8
K = 2
C = 5120
NC = 8
TPC = N // NC
H2 = 384  # task-head hidden dim (D // 2)
NEG_SLOPE = 0.2
TASK_DIMS = (1, 101, 1000)
TASK_DIMS_PAD = (4, 104, 1000)  # padded to mult-of-4 for PE fp32 free dims
DCH = D // 128  # 6 feature chunks
F32R_MM = True  # use float32r matmul mode

_last_results = None  # stashed BassKernelResults (for test harness inspection)


# ----------------------------------------------------------------------------
# host-side routing
# ----------------------------------------------------------------------------

def _softmax(x):
    m = x.max(axis=-1, keepdims=True)
    p = np.exp(x - m)
    return p / p.sum(axis=-1, keepdims=True)


def _route(nlp_pooled, task_in, task_emb, router_w):
    x = nlp_pooled.astype(np.float32)
    logits = (x + task_emb[task_in]) @ router_w
    probs = _softmax(logits)
    order = np.argsort(-probs, axis=-1, kind="stable")
    eidx = order[:, :K].astype(np.int64)
    gate = np.take_along_axis(probs, eidx, axis=-1)
    gate = gate / gate.sum(axis=-1, keepdims=True)

    ef = eidx.reshape(-1)
    pos = np.zeros(N * K, dtype=np.int64)
    for e in range(E):
        sel = ef == e
        pos[sel] = np.arange(sel.sum())
    keep = pos < C
    gflat = (gate.reshape(-1) * keep).reshape(N, K).astype(np.float32)

    oh = np.zeros((N, K, E), np.float32)
    for k in range(K):
        oh[np.arange(N), k, eidx[:, k]] = 1.0
    density = oh.sum(axis=1).mean(axis=0) / K
    aux = np.float32(E * np.sum(density * probs.mean(axis=0)))
    return eidx, gflat, keep, aux


def _assign_tokens(task_in, eidx):
    quota = np.zeros((NC, 3), np.int64)
    for i in range(3):
        cnt = int((task_in == i).sum())
        base, extra = divmod(cnt, NC)
        quota[:, i] = base
        quota[:extra, i] += 1
    ecnt = np.zeros((NC, E), np.int64)
    core_tokens = [[] for _ in range(NC)]
    taken = np.zeros((NC, 3), np.int64)
    for i in range(3):
        ids = np.nonzero(task_in == i)[0]
        for t in ids:
            e0, e1 = eidx[t]
            best, bestscore = -1, None
            for c in range(NC):
                if taken[c, i] >= quota[c, i]:
                    continue
                score = (
                    max(ecnt[c, e0], ecnt[c, e1]),
                    ecnt[c, e0] + ecnt[c, e1],
                    len(core_tokens[c]),
                )
                if bestscore is None or score < bestscore:
                    best, bestscore = c, score
            c = best
            taken[c, i] += 1
            ecnt[c, e0] += 1
            ecnt[c, e1] += 1
            core_tokens[c].append(t)
    # round block sizes up to a multiple of 4 (PE fp32 matmuls need even
    # moving-operand free dims; 4 keeps DMA aligned too)
    M = (quota.max(axis=0) + 3) // 4 * 4
    return [np.asarray(ct, dtype=np.int64) for ct in core_tokens], M, ecnt


def _build_metadata(nlp_pooled, task_in, task_emb, router_w):
    eidx, gate, keep, aux = _route(nlp_pooled, task_in, task_emb, router_w)
    core_tokens, M, ecnt = _assign_tokens(np.asarray(task_in), eidx)

    P_E = (int(ecnt.max()) + 7) // 8 * 8
    RT = E * P_E
    TT = (int(M.sum()) + 127) // 128 * 128
    offs = np.concatenate([[0], np.cumsum(M)]).astype(np.int64)

    x = nlp_pooled.astype(np.float32)
    task_in = np.asarray(task_in)
    per_core = []
    for c in range(NC):
        toks = core_tokens[c]
        tt = task_in[toks]
        lslots = np.full(TT, -1, dtype=np.int64)
        placed = {}
        for i in range(3):
            ids = np.sort(toks[tt == i])
            lslots[offs[i] : offs[i] + len(ids)] = ids
            placed[i] = ids
        xbufT = np.zeros((D, RT), np.float32)
        slot_row = np.zeros((2, TT), np.int64)
        gk = np.zeros((2, TT), np.float32)
        fill = np.zeros(E, np.int64)
        valid = lslots >= 0
        for ell in np.nonzero(valid)[0]:
            t = lslots[ell]
            for k in range(K):
                if keep[t * K + k]:
                    e = eidx[t, k]
                    r = e * P_E + fill[e]
                    fill[e] += 1
                    xbufT[:, r] = x[t]
                    slot_row[k, ell] = r
                    gk[k, ell] = gate[t, k]
        per_core.append(dict(xbufT=xbufT, slot_row=slot_row, gk=gk, placed=placed))
    meta = dict(P_E=P_E, RT=RT, TT=TT, M=M, offs=offs, aux=aux)
    return meta, per_core


# ----------------------------------------------------------------------------
# device kernel
# ----------------------------------------------------------------------------

def _chunks(total, step):
    out, n0 = [], 0
    while n0 < total:
        out.append((n0, min(step, total - n0)))
        n0 += step
    return out


def _build_bass(meta):
    import concourse.bacc as bacc
    import concourse.bass as bass
    import concourse.mybir as mybir
    import concourse.tile as tile
    from concourse.masks import make_identity

    f32 = mybir.dt.float32
    f32r = mybir.dt.float32r
    i32 = mybir.dt.int32
    AF = mybir.ActivationFunctionType
    ALU = mybir.AluOpType

    fmm = f32r if F32R_MM else f32

    def mmcast(ap):
        return ap

    P_E, RT, TT, M, offs = meta["P_E"], meta["RT"], meta["TT"], meta["M"], meta["offs"]
    JT = TT // 128  # combine chunks

    nc = bacc.Bacc("TRN2", target_bir_lowering=False, debug=False)

    # --- external IO ------------------------------------------------------
    xT_h = nc.dram_tensor("xbufT", [D, RT], fmm, kind="ExternalInput")
    sidx_h = nc.dram_tensor("sidx", [128, 2 * JT], i32, kind="ExternalInput")
    gates_h = nc.dram_tensor("gates", [128, 2 * JT], f32, kind="ExternalInput")
    w1_h = nc.dram_tensor("w1", [E, D, D], fmm, kind="ExternalInput")
    w2_h = nc.dram_tensor("w2", [E, D, D], fmm, kind="ExternalInput")
    b1r_h = nc.dram_tensor("b1r", [128, E * DCH], f32, kind="ExternalInput")
    b2bc_h = nc.dram_tensor("b2bc", [128, E * D], f32, kind="ExternalInput")
    w0_h = nc.dram_tensor("w0p", [3, D, H2], fmm, kind="ExternalInput")
    b0r_h = nc.dram_tensor("b0r", [128, 9], f32, kind="ExternalInput")
    w1p_h = nc.dram_tensor("w1p", [H2, sum(TASK_DIMS_PAD)], fmm, kind="ExternalInput")
    b1bc_h = nc.dram_tensor("b1bc", [128, sum(TASK_DIMS_PAD)], f32, kind="ExternalInput")
    out_h = [
        nc.dram_tensor(f"out{i}", [int(M[i]), TASK_DIMS[i]], f32, kind="ExternalOutput")
        for i in range(3)
    ]
    eo_h = nc.dram_tensor("eo_scratch", [RT, D], f32)  # internal scratch
    woffs = np.concatenate([[0], np.cumsum(TASK_DIMS_PAD)]).astype(np.int64)

    with tile.TileContext(nc) as tc:
        with (
            tc.tile_pool(name="const", bufs=1) as cp,
            tc.tile_pool(name="psum", bufs=1, space="PSUM") as pp,
        ):
            ident = cp.tile([128, 128], f32, tag="ident")
            make_identity(nc, ident[:])
            sidx = cp.tile([128, 2 * JT], i32, tag="sidx")
            nc.sync.dma_start(out=sidx[:], in_=sidx_h[:, :])
            gts = cp.tile([128, 2 * JT], f32, tag="gts")
            nc.sync.dma_start(out=gts[:], in_=gates_h[:, :])
            b1rt = cp.tile([128, E * DCH], f32, tag="b1rt")
            nc.sync.dma_start(out=b1rt[:], in_=b1r_h[:, :])
            b0rt = cp.tile([128, 9], f32, tag="b0rt")
            nc.sync.dma_start(out=b0rt[:], in_=b0r_h[:, :])

            # ------------- phase 1: expert FFN -------------
            with (
                tc.tile_pool(name="p1w", bufs=2) as wp,
                tc.tile_pool(name="p1x", bufs=2) as xp,
                tc.tile_pool(name="p1h", bufs=2) as hp,
                tc.tile_pool(name="p1eo", bufs=4) as ep,
                tc.tile_pool(name="p1c", bufs=1) as c1p,
            ):
                b2bct = c1p.tile([128, E * D], f32, tag="b2bct")
                nc.sync.dma_start(out=b2bct[:], in_=b2bc_h[:, :])
                for e in range(E):
                    w1t = []
                    w2t = []
                    xt = []
                    for d in range(DCH):
                        t = wp.tile([128, D], fmm, tag=f"w1_{d}", name=f"w1_{e}_{d}")
                        nc.sync.dma_start(
                            out=t[:], in_=w1_h[e, 128 * d : 128 * (d + 1), :]
                        )
                        w1t.append(t)
                    for d in range(DCH):
                        t = wp.tile([128, D], fmm, tag=f"w2_{d}", name=f"w2_{e}_{d}")
                        nc.sync.dma_start(
                            out=t[:], in_=w2_h[e, 128 * d : 128 * (d + 1), :]
                        )
                        w2t.append(t)
                    for d in range(DCH):
                        t = xp.tile([128, P_E], fmm, tag=f"x_{d}", name=f"x_{e}_{d}")
                        nc.sync.dma_start(
                            out=t[:],
                            in_=xT_h[
                                128 * d : 128 * (d + 1), e * P_E : (e + 1) * P_E
                            ],
                        )
                        xt.append(t)
                    # L1: hT[h] = relu(sum_d w1[d,h].T @ xT[d] + b1)
                    hT = [
                        hp.tile([128, P_E], fmm, tag=f"hT_{h}", name=f"hT_{e}_{h}")
                        for h in range(DCH)
                    ]
                    for h in range(DCH):
                        for n0, nw in _chunks(P_E, 512):
                            ps = pp.tile(
                                [128, 512], f32, tag="mm", bufs=4, name=f"ps1_{e}_{h}_{n0}"
                            )
                            for d in range(DCH):
                                nc.tensor.matmul(
                                    ps[:, :nw],
                                    lhsT=mmcast(w1t[d][:, 128 * h : 128 * (h + 1)]),
                                    rhs=mmcast(xt[d][:, n0 : n0 + nw]),
                                    start=(d == 0),
                                    stop=(d == DCH - 1),
                                )
                            nc.scalar.activation(
                                hT[h][:, n0 : n0 + nw],
                                ps[:, :nw],
                                AF.Relu,
                                bias=b1rt[:, DCH * e + h : DCH * e + h + 1],
                            )
                    # L2: eo[r-block] = sum_h hT[h][:, r].T @ w2[h] + b2
                    for r0, rw in _chunks(P_E, 128):
                        eos = ep.tile([128, D], f32, tag="eos", name=f"eos_{e}_{r0}")
                        for n0, nw in _chunks(D, 512):
                            ps = pp.tile(
                                [128, 512], f32, tag="mm", bufs=4,
                                name=f"ps2_{e}_{r0}_{n0}",
                            )
                            for h in range(DCH):
                                nc.tensor.matmul(
                                    ps[:rw, :nw],
                                    lhsT=mmcast(hT[h][:, r0 : r0 + rw]),
                                    rhs=mmcast(w2t[h][:, n0 : n0 + nw]),
                                    start=(h == 0),
                                    stop=(h == DCH - 1),
                                )
                            nc.vector.tensor_tensor(
                                out=eos[:rw, n0 : n0 + nw],
                                in0=ps[:rw, :nw],
                                in1=b2bct[:rw, D * e + n0 : D * e + n0 + nw],
                                op=ALU.add,
                            )
                        nc.sync.dma_start(
                            out=eo_h[e * P_E + r0 : e * P_E + r0 + rw, :],
                            in_=eos[:rw, :],
                        )

            # ------------- phase 2+3 pools -------------
            with (
                tc.tile_pool(name="p2y", bufs=1) as yp,
                tc.tile_pool(name="p2g", bufs=3) as gp,
                tc.tile_pool(name="p3w", bufs=2) as hwp,
                tc.tile_pool(name="p3c", bufs=1) as c3p,
            ):
                yT = [
                    yp.tile([128, TT], fmm, tag=f"yT{d}", name=f"yT{d}")
                    for d in range(DCH)
                ]
                # ------------- phase 2: combine + transpose -------------
                for j in range(JT):
                    ga = gp.tile([128, D], f32, tag="ga", name=f"ga_{j}")
                    gb = gp.tile([128, D], f32, tag="gb", name=f"gb_{j}")
                    nc.gpsimd.indirect_dma_start(
                        out=ga[:],
                        out_offset=None,
                        in_=eo_h[:, :],
                        in_offset=bass.IndirectOffsetOnAxis(
                            ap=sidx[:, j : j + 1], axis=0
                        ),
                    )
                    nc.gpsimd.indirect_dma_start(
                        out=gb[:],
                        out_offset=None,
                        in_=eo_h[:, :],
                        in_offset=bass.IndirectOffsetOnAxis(
                            ap=sidx[:, JT + j : JT + j + 1], axis=0
                        ),
                    )
                    yj = gp.tile([128, D], f32, tag="yj", name=f"yj_{j}")
                    nc.vector.tensor_scalar_mul(yj[:], ga[:], gts[:, j : j + 1])
                    gu = gp.tile([128, D], f32, tag="gu", name=f"gu_{j}")
                    nc.scalar.activation(
                        gu[:], gb[:], AF.Identity,
                        scale=gts[:, JT + j : JT + j + 1],
                    )
                    nc.vector.tensor_add(yj[:], yj[:], gu[:])
                    for d in range(DCH):
                        tp = pp.tile(
                            [128, 128], f32, tag="tp", bufs=2, name=f"tp_{j}_{d}"
                        )
                        nc.tensor.transpose(
                            tp[:], yj[:, 128 * d : 128 * (d + 1)], ident[:]
                        )
                        if d % 2 == 0:
                            nc.vector.tensor_copy(
                                yT[d][:, 128 * j : 128 * (j + 1)], tp[:]
                            )
                        else:
                            nc.scalar.copy(
                                yT[d][:, 128 * j : 128 * (j + 1)], tp[:]
                            )

                # ------------- phase 3: task heads -------------
                b1bct = c3p.tile([128, sum(TASK_DIMS_PAD)], f32, tag="b1bct")
                nc.sync.dma_start(out=b1bct[:], in_=b1bc_h[:, :])
                w1pt = []
                for h in range(H2 // 128):
                    t = c3p.tile([128, sum(TASK_DIMS_PAD)], fmm, tag=f"w1p_{h}")
                    nc.sync.dma_start(
                        out=t[:], in_=w1p_h[128 * h : 128 * (h + 1), :]
                    )
                    w1pt.append(t)
                for i in range(3):
                    Mi = int(M[i])
                    di = TASK_DIMS[i]
                    w0t = []
                    for d in range(DCH):
                        t = hwp.tile([128, H2], fmm, tag=f"w0_{d}", name=f"w0_{i}_{d}")
                        nc.sync.dma_start(
                            out=t[:], in_=w0_h[i, 128 * d : 128 * (d + 1), :]
                        )
                        w0t.append(t)
                    hh = [
                        hwp.tile(
                            [128, int(max(M))], fmm, tag=f"hh_{h}", name=f"hh_{i}_{h}"
                        )
                        for h in range(H2 // 128)
                    ]
                    for h in range(H2 // 128):
                        for n0, nw in _chunks(Mi, 512):
                            ps = pp.tile(
                                [128, 512], f32, tag="mm", bufs=4,
                                name=f"ps4_{i}_{h}_{n0}",
                            )
                            for d in range(DCH):
                                nc.tensor.matmul(
                                    ps[:, :nw],
                                    lhsT=mmcast(w0t[d][:, 128 * h : 128 * (h + 1)]),
                                    rhs=mmcast(
                                        yT[d][:, int(offs[i]) + n0 : int(offs[i]) + n0 + nw]
                                    ),
                                    start=(d == 0),
                                    stop=(d == DCH - 1),
                                )
                            # leaky_relu(v) = max(v, NEG_SLOPE * v), v = psum + b0
                            nc.scalar.activation(
                                hh[h][:, n0 : n0 + nw],
                                ps[:, :nw],
                                AF.Identity,
                                bias=b0rt[:, 3 * i + h : 3 * i + h + 1],
                            )
                            lk = gp.tile(
                                [128, 512], fmm, tag="lk", name=f"lk_{i}_{h}_{n0}"
                            )
                            nc.vector.tensor_scalar_mul(
                                lk[:, :nw], hh[h][:, n0 : n0 + nw], NEG_SLOPE
                            )
                            nc.vector.tensor_max(
                                hh[h][:, n0 : n0 + nw],
                                hh[h][:, n0 : n0 + nw],
                                lk[:, :nw],
                            )
                    for r0, rw in _chunks(Mi, 128):
                        for n0, nw in _chunks(TASK_DIMS_PAD[i], 512):
                            wout = min(nw, di - n0)
                            ps = pp.tile(
                                [128, 512], f32, tag="mm", bufs=4,
                                name=f"ps5_{i}_{r0}_{n0}",
                            )
                            for h in range(H2 // 128):
                                nc.tensor.matmul(
                                    ps[:rw, :nw],
                                    lhsT=mmcast(hh[h][:, r0 : r0 + rw]),
                                    rhs=mmcast(
                                        w1pt[h][:, int(woffs[i]) + n0 : int(woffs[i]) + n0 + nw]
                                    ),
                                    start=(h == 0),
                                    stop=(h == H2 // 128 - 1),
                                )
                            osb = gp.tile(
                                [128, 512], f32, tag="osb", name=f"osb_{i}_{r0}_{n0}"
                            )
                            nc.vector.tensor_tensor(
                                out=osb[:rw, :nw],
                                in0=ps[:rw, :nw],
                                in1=b1bct[:rw, int(woffs[i]) + n0 : int(woffs[i]) + n0 + nw],
                                op=ALU.add,
                            )
                            nc.sync.dma_start(
                                out=out_h[i][r0 : r0 + rw, n0 : n0 + wout],
                                in_=osb[:rw, :wout],
                            )
    nc.compile()
    return nc


# ----------------------------------------------------------------------------
# entry point
# ----------------------------------------------------------------------------

def kernel(**inputs):
    global _last_results
    from concourse import bass_utils

    nlp_pooled = np.asarray(inputs["nlp_pooled"], np.float32)
    task_in = np.asarray(inputs["task_in"], np.int32)
    task_emb = np.asarray(inputs["task_emb"], np.float32)
    router_w = np.asarray(inputs["router_w"], np.float32)
    w1 = np.ascontiguousarray(np.asarray(inputs["expert_w1"], np.float32))
    b1 = np.asarray(inputs["expert_b1"], np.float32)
    w2 = np.ascontiguousarray(np.asarray(inputs["expert_w2"], np.float32))
    b2 = np.asarray(inputs["expert_b2"], np.float32)

    meta, per_core = _build_metadata(nlp_pooled, task_in, task_emb, router_w)
    P_E, RT, TT, M, offs = meta["P_E"], meta["RT"], meta["TT"], meta["M"], meta["offs"]
    JT = TT // 128

    # shared (same for all cores) input arrays
    b1r = b1.reshape(E, DCH, 128).transpose(2, 0, 1).reshape(128, E * DCH)
    b1r = np.ascontiguousarray(b1r)
    b2bc = np.ascontiguousarray(np.broadcast_to(b2.reshape(1, E * D), (128, E * D)))
    w0p = np.stack([np.asarray(inputs[f"t{i}_w0"], np.float32) for i in range(3)])
    b0r = np.zeros((128, 9), np.float32)
    for i in range(3):
        b0 = np.asarray(inputs[f"t{i}_b0"], np.float32)
        b0r[:, 3 * i : 3 * i + 3] = b0.reshape(3, 128).T
    TDP = TASK_DIMS_PAD
    toffs = np.concatenate([[0], np.cumsum(TDP)]).astype(np.int64)
    w1p = np.zeros((H2, int(sum(TDP))), np.float32)
    b1p = np.zeros(int(sum(TDP)), np.float32)
    for i in range(3):
        w1p[:, toffs[i] : toffs[i] + TASK_DIMS[i]] = np.asarray(
            inputs[f"t{i}_w1"], np.float32
        )
        b1p[toffs[i] : toffs[i] + TASK_DIMS[i]] = np.asarray(
            inputs[f"t{i}_b1"], np.float32
        )
    b1bc = np.ascontiguousarray(np.broadcast_to(b1p.reshape(1, -1), (128, b1p.size)))

    shared = dict(
        w1=w1, w2=w2, b1r=b1r, b2bc=b2bc,
        w0p=np.ascontiguousarray(w0p), b0r=b0r,
        w1p=np.ascontiguousarray(w1p), b1bc=b1bc,
    )

    in_maps = []
    for c in range(NC):
        pc = per_core[c]
        sidx = np.zeros((128, 2 * JT), np.int32)
        gates = np.zeros((128, 2 * JT), np.float32)
        for k in range(2):
            sidx[:, k * JT : (k + 1) * JT] = (
                pc["slot_row"][k].reshape(JT, 128).T
            )
            gates[:, k * JT : (k + 1) * JT] = pc["gk"][k].reshape(JT, 128).T
        in_maps.append(
            dict(xbufT=pc["xbufT"], sidx=sidx, gates=gates, **shared)
        )

    nc = _build_bass(meta)
    results = bass_utils.run_bass_kernel_spmd(
        nc, in_maps, core_ids=list(range(NC)),
        trace=bool(os.environ.get("BASS_TRACE")),
    )
    _last_results = results

    # assemble outputs
    idx_full = []
    rank = np.zeros(N, np.int64)
    for i in range(3):
        ids = np.nonzero(task_in == i)[0]
        rank[ids] = np.arange(len(ids))
        pad = np.full(N, -1, np.int64)
        pad[: len(ids)] = ids
        idx_full.append(pad.astype(np.int32))

    outs = [np.zeros((N, d), np.float32) for d in TASK_DIMS]
    for c in range(NC):
        pc = per_core[c]
        res = results.results[c]
        for i in range(3):
            ids = pc["placed"][i]
            if len(ids):
                outs[i][rank[ids]] = res[f"out{i}"][: len(ids)]

    return tuple(idx_full), tuple(outs), meta["aux"]


# revision 9
# speedup vs baseline: 1.0923x; 1.0923x over previous
"""MoE routing + task-head kernel for 8 Trainium2 NeuronCores.

Strategy (self-contained, shapes hardcoded from the problem):
  - Host (numpy): router softmax/top-2, capacity positions, gates, aux loss,
    and a balanced token->core assignment (2048 tokens/core) that equalizes
    per-core per-task counts and per-(core,expert) slot counts.  The host
    pre-gathers each core's tokens into an expert-grouped, feature-major
    dispatch buffer xbufT [D, E*P_E].
  - Device (8-way SPMD, one NEFF): per core
      phase 1: per-expert FFN  hT = relu(W1^T-tiles @ xT + b1),
               eo = hT-tiles @ W2 + b2  -> DRAM [E*P_E, D] (row-major)
      phase 2: combine  y = g0*eo[slot0] + g1*eo[slot1] via indirect row
               gathers, then PE-transpose y -> yT [D, TT]
      phase 3: task heads  o_i = lrelu(yT-block @ w0_i + b0_i) @ w1_i + b1_i
  - Host: scatter per-core head outputs back to global row order, return
    (task_indices, task_outs, aux_loss) exactly like the reference.

All matmuls run as float32r (full-rate PE mode, fp32 storage).
"""

import os

import numpy as np

N = 16384
D = 768
E = 8
K = 2
C = 5120
NC = 8
TPC = N // NC
H2 = 384  # task-head hidden dim (D // 2)
NEG_SLOPE = 0.2
TASK_DIMS = (1, 101, 1000)
TASK_DIMS_PAD = (4, 104, 1000)  # padded to mult-of-4 for PE fp32 free dims
DCH = D // 128  # 6 feature chunks
F32R_MM = True  # use float32r matmul mode

_last_results = None  # stashed BassKernelResults (for test harness inspection)


# ----------------------------------------------------------------------------
# host-side routing
# ----------------------------------------------------------------------------

def _softmax(x):
    m = x.max(axis=-1, keepdims=True)
    p = np.exp(x - m)
    return p / p.sum(axis=-1, keepdims=True)


def _route(nlp_pooled, task_in, task_emb, router_w):
    x = nlp_pooled.astype(np.float32)
    logits = (x + task_emb[task_in]) @ router_w
    probs = _softmax(logits)
    order = np.argsort(-probs, axis=-1, kind="stable")
    eidx = order[:, :K].astype(np.int64)
    gate = np.take_along_axis(probs, eidx, axis=-1)
    gate = gate / gate.sum(axis=-1, keepdims=True)

    ef = eidx.reshape(-1)
    pos = np.zeros(N * K, dtype=np.int64)
    for e in range(E):
        sel = ef == e
        pos[sel] = np.arange(sel.sum())
    keep = pos < C
    gflat = (gate.reshape(-1) * keep).reshape(N, K).astype(np.float32)

    oh = np.zeros((N, K, E), np.float32)
    for k in range(K):
        oh[np.arange(N), k, eidx[:, k]] = 1.0
    density = oh.sum(axis=1).mean(axis=0) / K
    aux = np.float32(E * np.sum(density * probs.mean(axis=0)))
    return eidx, gflat, keep, aux


def _assign_tokens(task_in, eidx):
    quota = np.zeros((NC, 3), np.int64)
    for i in range(3):
        cnt = int((task_in == i).sum())
        base, extra = divmod(cnt, NC)
        quota[:, i] = base
        quota[:extra, i] += 1
    ecnt = np.zeros((NC, E), np.int64)
    core_tokens = [[] for _ in range(NC)]
    taken = np.zeros((NC, 3), np.int64)
    for i in range(3):
        ids = np.nonzero(task_in == i)[0]
        for t in ids:
            e0, e1 = eidx[t]
            best, bestscore = -1, None
            for c in range(NC):
                if taken[c, i] >= quota[c, i]:
                    continue
                score = (
                    max(ecnt[c, e0], ecnt[c, e1]),
                    ecnt[c, e0] + ecnt[c, e1],
                    len(core_tokens[c]),
                )
                if bestscore is None or score < bestscore:
                    best, bestscore = c, score
            c = best
            taken[c, i] += 1
            ecnt[c, e0] += 1
            ecnt[c, e1] += 1
            core_tokens[c].append(t)
    # round block sizes up to a multiple of 4 (PE fp32 matmuls need even
    # moving-operand free dims; 4 keeps DMA aligned too)
    M = (quota.max(axis=0) + 3) // 4 * 4
    return [np.asarray(ct, dtype=np.int64) for ct in core_tokens], M, ecnt


def _build_metadata(nlp_pooled, task_in, task_emb, router_w):
    eidx, gate, keep, aux = _route(nlp_pooled, task_in, task_emb, router_w)
    core_tokens, M, ecnt = _assign_tokens(np.asarray(task_in), eidx)

    P_E = (int(ecnt.max()) + 7) // 8 * 8
    RT = E * P_E
    TT = (int(M.sum()) + 127) // 128 * 128
    offs = np.concatenate([[0], np.cumsum(M)]).astype(np.int64)

    x = nlp_pooled.astype(np.float32)
    task_in = np.asarray(task_in)
    per_core = []
    for c in range(NC):
        toks = core_tokens[c]
        tt = task_in[toks]
        lslots = np.full(TT, -1, dtype=np.int64)
        placed = {}
        for i in range(3):
            ids = np.sort(toks[tt == i])
            lslots[offs[i] : offs[i] + len(ids)] = ids
            placed[i] = ids
        xbufT = np.zeros((D, RT), np.float32)
        slot_row = np.zeros((2, TT), np.int64)
        gk = np.zeros((2, TT), np.float32)
        fill = np.zeros(E, np.int64)
        valid = lslots >= 0
        for ell in np.nonzero(valid)[0]:
            t = lslots[ell]
            for k in range(K):
                if keep[t * K + k]:
                    e = eidx[t, k]
                    r = e * P_E + fill[e]
                    fill[e] += 1
                    xbufT[:, r] = x[t]
                    slot_row[k, ell] = r
                    gk[k, ell] = gate[t, k]
        per_core.append(dict(xbufT=xbufT, slot_row=slot_row, gk=gk, placed=placed))
    meta = dict(P_E=P_E, RT=RT, TT=TT, M=M, offs=offs, aux=aux)
    return meta, per_core


# ----------------------------------------------------------------------------
# device kernel
# ----------------------------------------------------------------------------

def _chunks(total, step):
    out, n0 = [], 0
    while n0 < total:
        out.append((n0, min(step, total - n0)))
        n0 += step
    return out


def _build_bass(meta):
    import concourse.bacc as bacc
    import concourse.bass as bass
    import concourse.mybir as mybir
    import concourse.tile as tile
    from concourse.masks import make_identity

    f32 = mybir.dt.float32
    f32r = mybir.dt.float32r
    i32 = mybir.dt.int32
    AF = mybir.ActivationFunctionType
    ALU = mybir.AluOpType

    fmm = f32r if F32R_MM else f32

    def mmcast(ap):
        return ap

    P_E, RT, TT, M, offs = meta["P_E"], meta["RT"], meta["TT"], meta["M"], meta["offs"]
    JT = TT // 128  # combine chunks

    nc = bacc.Bacc("TRN2", target_bir_lowering=False, debug=False)

    # --- external IO ------------------------------------------------------
    xT_h = nc.dram_tensor("xbufT", [D, RT], fmm, kind="ExternalInput")
    sidx_h = nc.dram_tensor("sidx", [128, 2 * JT], i32, kind="ExternalInput")
    gates_h = nc.dram_tensor("gates", [128, 2 * JT], f32, kind="ExternalInput")
    w1_h = nc.dram_tensor("w1", [E, D, D], fmm, kind="ExternalInput")
    w2_h = nc.dram_tensor("w2", [E, D, D], fmm, kind="ExternalInput")
    b1r_h = nc.dram_tensor("b1r", [128, E * DCH], f32, kind="ExternalInput")
    b2bc_h = nc.dram_tensor("b2bc", [128, E * D], f32, kind="ExternalInput")
    w0_h = nc.dram_tensor("w0p", [3, D, H2], fmm, kind="ExternalInput")
    b0r_h = nc.dram_tensor("b0r", [128, 9], f32, kind="ExternalInput")
    w1p_h = nc.dram_tensor("w1p", [H2, sum(TASK_DIMS_PAD)], fmm, kind="ExternalInput")
    b1bc_h = nc.dram_tensor("b1bc", [128, sum(TASK_DIMS_PAD)], f32, kind="ExternalInput")
    out_h = [
        nc.dram_tensor(f"out{i}", [int(M[i]), TASK_DIMS[i]], f32, kind="ExternalOutput")
        for i in range(3)
    ]
    eo_h = nc.dram_tensor("eo_scratch", [RT, D], f32)  # internal scratch
    woffs = np.concatenate([[0], np.cumsum(TASK_DIMS_PAD)]).astype(np.int64)

    with tile.TileContext(nc) as tc:
        with (
            tc.tile_pool(name="const", bufs=1) as cp,
            tc.tile_pool(name="psum", bufs=1, space="PSUM") as pp,
        ):
            ident = cp.tile([128, 128], f32, tag="ident")
            make_identity(nc, ident[:])
            sidx = cp.tile([128, 2 * JT], i32, tag="sidx")
            nc.sync.dma_start(out=sidx[:], in_=sidx_h[:, :])
            gts = cp.tile([128, 2 * JT], f32, tag="gts")
            nc.sync.dma_start(out=gts[:], in_=gates_h[:, :])
            b1rt = cp.tile([128, E * DCH], f32, tag="b1rt")
            nc.sync.dma_start(out=b1rt[:], in_=b1r_h[:, :])
            b0rt = cp.tile([128, 9], f32, tag="b0rt")
            nc.sync.dma_start(out=b0rt[:], in_=b0r_h[:, :])

            # ------------- phase 1: expert FFN -------------
            with (
                tc.tile_pool(name="p1w", bufs=2) as wp,
                tc.tile_pool(name="p1x", bufs=2) as xp,
                tc.tile_pool(name="p1h", bufs=2) as hp,
                tc.tile_pool(name="p1eo", bufs=4) as ep,
                tc.tile_pool(name="p1c", bufs=1) as c1p,
            ):
                b2bct = c1p.tile([128, E * D], f32, tag="b2bct")
                nc.scalar.dma_start(out=b2bct[:], in_=b2bc_h[:, :])
                for e in range(E):
                    w1t = []
                    w2t = []
                    xt = []
                    for d in range(DCH):
                        t = wp.tile([128, D], fmm, tag=f"w1_{d}", name=f"w1_{e}_{d}")
                        nc.sync.dma_start(
                            out=t[:], in_=w1_h[e, 128 * d : 128 * (d + 1), :]
                        )
                        w1t.append(t)
                    for d in range(DCH):
                        t = wp.tile([128, D], fmm, tag=f"w2_{d}", name=f"w2_{e}_{d}")
                        nc.scalar.dma_start(
                            out=t[:], in_=w2_h[e, 128 * d : 128 * (d + 1), :]
                        )
                        w2t.append(t)
                    for d in range(DCH):
                        t = xp.tile([128, P_E], fmm, tag=f"x_{d}", name=f"x_{e}_{d}")
                        nc.sync.dma_start(
                            out=t[:],
                            in_=xT_h[
                                128 * d : 128 * (d + 1), e * P_E : (e + 1) * P_E
                            ],
                        )
                        xt.append(t)
                    # L1: hT[h] = relu(sum_d w1[d,h].T @ xT[d] + b1)
                    hT = [
                        hp.tile([128, P_E], fmm, tag=f"hT_{h}", name=f"hT_{e}_{h}")
                        for h in range(DCH)
                    ]
                    for h in range(DCH):
                        for n0, nw in _chunks(P_E, 512):
                            ps = pp.tile(
                                [128, 512], f32, tag="mm", bufs=4, name=f"ps1_{e}_{h}_{n0}"
                            )
                            for d in range(DCH):
                                nc.tensor.matmul(
                                    ps[:, :nw],
                                    lhsT=mmcast(w1t[d][:, 128 * h : 128 * (h + 1)]),
                                    rhs=mmcast(xt[d][:, n0 : n0 + nw]),
                                    start=(d == 0),
                                    stop=(d == DCH - 1),
                                )
                            nc.scalar.activation(
                                hT[h][:, n0 : n0 + nw],
                                ps[:, :nw],
                                AF.Relu,
                                bias=b1rt[:, DCH * e + h : DCH * e + h + 1],
                            )
                    # L2: eo[r-block] = sum_h hT[h][:, r].T @ w2[h] + b2
                    for r0, rw in _chunks(P_E, 128):
                        eos = ep.tile([128, D], f32, tag="eos", name=f"eos_{e}_{r0}")
                        for n0, nw in _chunks(D, 512):
                            ps = pp.tile(
                                [128, 512], f32, tag="mm", bufs=4,
                                name=f"ps2_{e}_{r0}_{n0}",
                            )
                            for h in range(DCH):
                                nc.tensor.matmul(
                                    ps[:rw, :nw],
                                    lhsT=mmcast(hT[h][:, r0 : r0 + rw]),
                                    rhs=mmcast(w2t[h][:, n0 : n0 + nw]),
                                    start=(h == 0),
                                    stop=(h == DCH - 1),
                                )
                            nc.vector.tensor_tensor(
                                out=eos[:rw, n0 : n0 + nw],
                                in0=ps[:rw, :nw],
                                in1=b2bct[:rw, D * e + n0 : D * e + n0 + nw],
                                op=ALU.add,
                            )
                        nc.scalar.dma_start(
                            out=eo_h[e * P_E + r0 : e * P_E + r0 + rw, :],
                            in_=eos[:rw, :],
                        )

            # ------------- phase 2+3 pools -------------
            with (
                tc.tile_pool(name="p2y", bufs=1) as yp,
                tc.tile_pool(name="p2g", bufs=3) as gp,
                tc.tile_pool(name="p3w", bufs=2) as hwp,
                tc.tile_pool(name="p3c", bufs=1) as c3p,
            ):
                yT = [
                    yp.tile([128, TT], fmm, tag=f"yT{d}", name=f"yT{d}")
                    for d in range(DCH)
                ]
                # ------------- phase 2+3 interleaved -------------
                b1bct = c3p.tile([128, sum(TASK_DIMS_PAD)], f32, tag="b1bct")
                nc.scalar.dma_start(out=b1bct[:], in_=b1bc_h[:, :])
                w1pt = []
                for h in range(H2 // 128):
                    t = c3p.tile([128, sum(TASK_DIMS_PAD)], fmm, tag=f"w1p_{h}")
                    nc.scalar.dma_start(
                        out=t[:], in_=w1p_h[128 * h : 128 * (h + 1), :]
                    )
                    w1pt.append(t)

                def combine_chunk(j):
                    ga = gp.tile([128, D], f32, tag="ga", name=f"ga_{j}")
                    gb = gp.tile([128, D], f32, tag="gb", name=f"gb_{j}")
                    nc.gpsimd.indirect_dma_start(
                        out=ga[:],
                        out_offset=None,
                        in_=eo_h[:, :],
                        in_offset=bass.IndirectOffsetOnAxis(
                            ap=sidx[:, j : j + 1], axis=0
                        ),
                    )
                    nc.gpsimd.indirect_dma_start(
                        out=gb[:],
                        out_offset=None,
                        in_=eo_h[:, :],
                        in_offset=bass.IndirectOffsetOnAxis(
                            ap=sidx[:, JT + j : JT + j + 1], axis=0
                        ),
                    )
                    yj = gp.tile([128, D], f32, tag="yj", name=f"yj_{j}")
                    nc.vector.tensor_scalar_mul(yj[:], ga[:], gts[:, j : j + 1])
                    gu = gp.tile([128, D], f32, tag="gu", name=f"gu_{j}")
                    nc.scalar.activation(
                        gu[:], gb[:], AF.Identity,
                        scale=gts[:, JT + j : JT + j + 1],
                    )
                    nc.vector.tensor_add(yj[:], yj[:], gu[:])
                    for d in range(DCH):
                        tp = pp.tile(
                            [128, 128], f32, tag="tp", bufs=2, name=f"tp_{j}_{d}"
                        )
                        nc.tensor.transpose(
                            tp[:], yj[:, 128 * d : 128 * (d + 1)], ident[:]
                        )
                        if d % 2 == 0:
                            nc.vector.tensor_copy(
                                yT[d][:, 128 * j : 128 * (j + 1)], tp[:]
                            )
                        else:
                            nc.scalar.copy(
                                yT[d][:, 128 * j : 128 * (j + 1)], tp[:]
                            )

                jdone = 0
                for i in range(3):
                    Mi = int(M[i])
                    di = TASK_DIMS[i]
                    jend = (int(offs[i]) + Mi - 1) // 128
                    while jdone <= min(jend, JT - 1):
                        combine_chunk(jdone)
                        jdone += 1
                    w0t = []
                    for d in range(DCH):
                        t = hwp.tile([128, H2], fmm, tag=f"w0_{d}", name=f"w0_{i}_{d}")
                        nc.scalar.dma_start(
                            out=t[:], in_=w0_h[i, 128 * d : 128 * (d + 1), :]
                        )
                        w0t.append(t)
                    hh = [
                        hwp.tile(
                            [128, int(max(M))], fmm, tag=f"hh_{h}", name=f"hh_{i}_{h}"
                        )
                        for h in range(H2 // 128)
                    ]
                    for h in range(H2 // 128):
                        for n0, nw in _chunks(Mi, 512):
                            ps = pp.tile(
                                [128, 512], f32, tag="mm", bufs=4,
                                name=f"ps4_{i}_{h}_{n0}",
                            )
                            for d in range(DCH):
                                nc.tensor.matmul(
                                    ps[:, :nw],
                                    lhsT=mmcast(w0t[d][:, 128 * h : 128 * (h + 1)]),
                                    rhs=mmcast(
                                        yT[d][:, int(offs[i]) + n0 : int(offs[i]) + n0 + nw]
                                    ),
                                    start=(d == 0),
                                    stop=(d == DCH - 1),
                                )
                            # leaky_relu(v) = max(v, NEG_SLOPE * v), v = psum + b0
                            nc.scalar.activation(
                                hh[h][:, n0 : n0 + nw],
                                ps[:, :nw],
                                AF.Identity,
                                bias=b0rt[:, 3 * i + h : 3 * i + h + 1],
                            )
                            lk = gp.tile(
                                [128, 512], fmm, tag="lk", name=f"lk_{i}_{h}_{n0}"
                            )
                            nc.vector.tensor_scalar_mul(
                                lk[:, :nw], hh[h][:, n0 : n0 + nw], NEG_SLOPE
                            )
                            nc.vector.tensor_max(
                                hh[h][:, n0 : n0 + nw],
                                hh[h][:, n0 : n0 + nw],
                                lk[:, :nw],
                            )
                    for r0, rw in _chunks(Mi, 128):
                        for n0, nw in _chunks(TASK_DIMS_PAD[i], 512):
                            wout = min(nw, di - n0)
                            ps = pp.tile(
                                [128, 512], f32, tag="mm", bufs=4,
                                name=f"ps5_{i}_{r0}_{n0}",
                            )
                            for h in range(H2 // 128):
                                nc.tensor.matmul(
                                    ps[:rw, :nw],
                                    lhsT=mmcast(hh[h][:, r0 : r0 + rw]),
                                    rhs=mmcast(
                                        w1pt[h][:, int(woffs[i]) + n0 : int(woffs[i]) + n0 + nw]
                                    ),
                                    start=(h == 0),
                                    stop=(h == H2 // 128 - 1),
                                )
                            osb = gp.tile(
                                [128, 512], f32, tag="osb", name=f"osb_{i}_{r0}_{n0}"
                            )
                            nc.vector.tensor_tensor(
                                out=osb[:rw, :nw],
                                in0=ps[:rw, :nw],
                                in1=b1bct[:rw, int(woffs[i]) + n0 : int(woffs[i]) + n0 + nw],
                                op=ALU.add,
                            )
                            nc.sync.dma_start(
                                out=out_h[i][r0 : r0 + rw, n0 : n0 + wout],
                                in_=osb[:rw, :wout],
                            )
    nc.compile()
    return nc


# ----------------------------------------------------------------------------
# entry point
# ----------------------------------------------------------------------------

def kernel(**inputs):
    global _last_results
    from concourse import bass_utils

    nlp_pooled = np.asarray(inputs["nlp_pooled"], np.float32)
    task_in = np.asarray(inputs["task_in"], np.int32)
    task_emb = np.asarray(inputs["task_emb"], np.float32)
    router_w = np.asarray(inputs["router_w"], np.float32)
    w1 = np.ascontiguousarray(np.asarray(inputs["expert_w1"], np.float32))
    b1 = np.asarray(inputs["expert_b1"], np.float32)
    w2 = np.ascontiguousarray(np.asarray(inputs["expert_w2"], np.float32))
    b2 = np.asarray(inputs["expert_b2"], np.float32)

    meta, per_core = _build_metadata(nlp_pooled, task_in, task_emb, router_w)
    P_E, RT, TT, M, offs = meta["P_E"], meta["RT"], meta["TT"], meta["M"], meta["offs"]
    JT = TT // 128

    # shared (same for all cores) input arrays
    b1r = b1.reshape(E, DCH, 128).transpose(2, 0, 1).reshape(128, E * DCH)
    b1r = np.ascontiguousarray(b1r)
    b2bc = np.ascontiguousarray(np.broadcast_to(b2.reshape(1, E * D), (128, E * D)))
    w0p = np.stack([np.asarray(inputs[f"t{i}_w0"], np.float32) for i in range(3)])
    b0r = np.zeros((128, 9), np.float32)
    for i in range(3):
        b0 = np.asarray(inputs[f"t{i}_b0"], np.float32)
        b0r[:, 3 * i : 3 * i + 3] = b0.reshape(3, 128).T
    TDP = TASK_DIMS_PAD
    toffs = np.concatenate([[0], np.cumsum(TDP)]).astype(np.int64)
    w1p = np.zeros((H2, int(sum(TDP))), np.float32)
    b1p = np.zeros(int(sum(TDP)), np.float32)
    for i in range(3):
        w1p[:, toffs[i] : toffs[i] + TASK_DIMS[i]] = np.asarray(
            inputs[f"t{i}_w1"], np.float32
        )
        b1p[toffs[i] : toffs[i] + TASK_DIMS[i]] = np.asarray(
            inputs[f"t{i}_b1"], np.float32
        )
    b1bc = np.ascontiguousarray(np.broadcast_to(b1p.reshape(1, -1), (128, b1p.size)))

    shared = dict(
        w1=w1, w2=w2, b1r=b1r, b2bc=b2bc,
        w0p=np.ascontiguousarray(w0p), b0r=b0r,
        w1p=np.ascontiguousarray(w1p), b1bc=b1bc,
    )

    in_maps = []
    for c in range(NC):
        pc = per_core[c]
        sidx = np.zeros((128, 2 * JT), np.int32)
        gates = np.zeros((128, 2 * JT), np.float32)
        for k in range(2):
            sidx[:, k * JT : (k + 1) * JT] = (
                pc["slot_row"][k].reshape(JT, 128).T
            )
            gates[:, k * JT : (k + 1) * JT] = pc["gk"][k].reshape(JT, 128).T
        in_maps.append(
            dict(xbufT=pc["xbufT"], sidx=sidx, gates=gates, **shared)
        )

    nc = _build_bass(meta)
    results = bass_utils.run_bass_kernel_spmd(
        nc, in_maps, core_ids=list(range(NC)),
        trace=bool(os.environ.get("BASS_TRACE")),
    )
    _last_results = results

    # assemble outputs
    idx_full = []
    rank = np.zeros(N, np.int64)
    for i in range(3):
        ids = np.nonzero(task_in == i)[0]
        rank[ids] = np.arange(len(ids))
        pad = np.full(N, -1, np.int64)
        pad[: len(ids)] = ids
        idx_full.append(pad.astype(np.int32))

    outs = [np.zeros((N, d), np.float32) for d in TASK_DIMS]
    for c in range(NC):
        pc = per_core[c]
        res = results.results[c]
        for i in range(3):
            ids = pc["placed"][i]
            if len(ids):
                outs[i][rank[ids]] = res[f"out{i}"][: len(ids)]

    return tuple(idx_full), tuple(outs), meta["aux"]


# revision 12
# speedup vs baseline: 1.1798x; 1.0801x over previous
"""MoE routing + task-head kernel for 8 Trainium2 NeuronCores.

Strategy (self-contained, shapes hardcoded from the problem):
  - Host (numpy): router softmax/top-2, capacity positions, gates, aux loss,
    and a balanced token->core assignment (2048 tokens/core) that equalizes
    per-core per-task counts and per-(core,expert) slot counts.  The host
    pre-gathers each core's tokens into an expert-grouped, feature-major
    dispatch buffer xbufT [D, E*P_E].
  - Device (8-way SPMD, one NEFF): per core
      phase 1: per-expert FFN  hT = relu(W1^T-tiles @ xT + b1),
               eo = hT-tiles @ W2 + b2  -> DRAM [E*P_E, D] (row-major)
      phase 2: combine  y = g0*eo[slot0] + g1*eo[slot1] via indirect row
               gathers, then PE-transpose y -> yT [D, TT]
      phase 3: task heads  o_i = lrelu(yT-block @ w0_i + b0_i) @ w1_i + b1_i
  - Host: scatter per-core head outputs back to global row order, return
    (task_indices, task_outs, aux_loss) exactly like the reference.

All matmuls run as float32r (full-rate PE mode, fp32 storage).
"""

import os

import numpy as np

N = 16384
D = 768
E = 8
K = 2
C = 5120
NC = 8
TPC = N // NC
H2 = 384  # task-head hidden dim (D // 2)
NEG_SLOPE = 0.2
TASK_DIMS = (1, 101, 1000)
TASK_DIMS_PAD = (4, 104, 1000)  # padded to mult-of-4 for PE fp32 free dims
DCH = D // 128  # 6 feature chunks
F32R_MM = True  # use float32r matmul mode

_last_results = None  # stashed BassKernelResults (for test harness inspection)


# ----------------------------------------------------------------------------
# host-side routing
# ----------------------------------------------------------------------------

def _softmax(x):
    m = x.max(axis=-1, keepdims=True)
    p = np.exp(x - m)
    return p / p.sum(axis=-1, keepdims=True)


def _route(nlp_pooled, task_in, task_emb, router_w):
    x = nlp_pooled.astype(np.float32)
    logits = (x + task_emb[task_in]) @ router_w
    probs = _softmax(logits)
    order = np.argsort(-probs, axis=-1, kind="stable")
    eidx = order[:, :K].astype(np.int64)
    gate = np.take_along_axis(probs, eidx, axis=-1)
    gate = gate / gate.sum(axis=-1, keepdims=True)

    ef = eidx.reshape(-1)
    pos = np.zeros(N * K, dtype=np.int64)
    for e in range(E):
        sel = ef == e
        pos[sel] = np.arange(sel.sum())
    keep = pos < C
    gflat = (gate.reshape(-1) * keep).reshape(N, K).astype(np.float32)

    oh = np.zeros((N, K, E), np.float32)
    for k in range(K):
        oh[np.arange(N), k, eidx[:, k]] = 1.0
    density = oh.sum(axis=1).mean(axis=0) / K
    aux = np.float32(E * np.sum(density * probs.mean(axis=0)))
    return eidx, gflat, keep, aux


def _assign_tokens(task_in, eidx):
    quota = np.zeros((NC, 3), np.int64)
    for i in range(3):
        cnt = int((task_in == i).sum())
        base, extra = divmod(cnt, NC)
        quota[:, i] = base
        quota[:extra, i] += 1
    ecnt = np.zeros((NC, E), np.int64)
    core_tokens = [[] for _ in range(NC)]
    taken = np.zeros((NC, 3), np.int64)
    for i in range(3):
        ids = np.nonzero(task_in == i)[0]
        for t in ids:
            e0, e1 = eidx[t]
            best, bestscore = -1, None
            for c in range(NC):
                if taken[c, i] >= quota[c, i]:
                    continue
                score = (
                    max(ecnt[c, e0], ecnt[c, e1]),
                    ecnt[c, e0] + ecnt[c, e1],
                    len(core_tokens[c]),
                )
                if bestscore is None or score < bestscore:
                    best, bestscore = c, score
            c = best
            taken[c, i] += 1
            ecnt[c, e0] += 1
            ecnt[c, e1] += 1
            core_tokens[c].append(t)
    # round block sizes up to a multiple of 4 (PE fp32 matmuls need even
    # moving-operand free dims; 4 keeps DMA aligned too)
    M = (quota.max(axis=0) + 3) // 4 * 4
    return [np.asarray(ct, dtype=np.int64) for ct in core_tokens], M, ecnt


def _build_metadata(nlp_pooled, task_in, task_emb, router_w):
    eidx, gate, keep, aux = _route(nlp_pooled, task_in, task_emb, router_w)
    core_tokens, M, ecnt = _assign_tokens(np.asarray(task_in), eidx)

    P_E = (int(ecnt.max()) + 7) // 8 * 8
    RT = E * P_E
    TT = (int(M.sum()) + 127) // 128 * 128
    offs = np.concatenate([[0], np.cumsum(M)]).astype(np.int64)

    x = nlp_pooled.astype(np.float32)
    task_in = np.asarray(task_in)
    per_core = []
    for c in range(NC):
        toks = core_tokens[c]
        tt = task_in[toks]
        lslots = np.full(TT, -1, dtype=np.int64)
        placed = {}
        for i in range(3):
            ids = np.sort(toks[tt == i])
            lslots[offs[i] : offs[i] + len(ids)] = ids
            placed[i] = ids
        xbufT = np.zeros((D, RT), np.float32)
        slot_row = np.zeros((2, TT), np.int64)
        gk = np.zeros((2, TT), np.float32)
        fill = np.zeros(E, np.int64)
        valid = lslots >= 0
        for ell in np.nonzero(valid)[0]:
            t = lslots[ell]
            for k in range(K):
                if keep[t * K + k]:
                    e = eidx[t, k]
                    r = e * P_E + fill[e]
                    fill[e] += 1
                    xbufT[:, r] = x[t]
                    slot_row[k, ell] = r
                    gk[k, ell] = gate[t, k]
        per_core.append(dict(xbufT=xbufT, slot_row=slot_row, gk=gk, placed=placed))
    meta = dict(P_E=P_E, RT=RT, TT=TT, M=M, offs=offs, aux=aux)
    return meta, per_core


# ----------------------------------------------------------------------------
# device kernel
# ----------------------------------------------------------------------------

def _chunks(total, step):
    out, n0 = [], 0
    while n0 < total:
        out.append((n0, min(step, total - n0)))
        n0 += step
    return out


def _chunks_bal(total, maxstep):
    """Near-equal chunks (each a multiple of 4, <= maxstep) so no chunk is a
    tiny ragged tail whose matmul can't hide its LDWEIGHTS."""
    nch = -(-total // maxstep)
    step = (-(-total // nch) + 3) // 4 * 4
    return _chunks(total, step)


def _build_bass(meta):
    import concourse.bacc as bacc
    import concourse.bass as bass
    import concourse.mybir as mybir
    import concourse.tile as tile
    from concourse.masks import make_identity

    f32 = mybir.dt.float32
    f32r = mybir.dt.float32r
    i32 = mybir.dt.int32
    AF = mybir.ActivationFunctionType
    ALU = mybir.AluOpType

    fmm = f32r if F32R_MM else f32

    def mmcast(ap):
        return ap

    P_E, RT, TT, M, offs = meta["P_E"], meta["RT"], meta["TT"], meta["M"], meta["offs"]
    JT = TT // 128  # combine chunks

    nc = bacc.Bacc("TRN2", target_bir_lowering=False, debug=False)

    # --- external IO ------------------------------------------------------
    xT_h = nc.dram_tensor("xbufT", [D, RT], fmm, kind="ExternalInput")
    sidx_h = nc.dram_tensor("sidx", [128, 2 * JT], i32, kind="ExternalInput")
    gates_h = nc.dram_tensor("gates", [128, 2 * JT], f32, kind="ExternalInput")
    w1_h = nc.dram_tensor("w1", [E, D, D], fmm, kind="ExternalInput")
    w2_h = nc.dram_tensor("w2", [E, D, D], fmm, kind="ExternalInput")
    b1r_h = nc.dram_tensor("b1r", [128, E * DCH], f32, kind="ExternalInput")
    b2bc_h = nc.dram_tensor("b2bc", [128, E * D], f32, kind="ExternalInput")
    w0_h = nc.dram_tensor("w0p", [3, D, H2], fmm, kind="ExternalInput")
    b0r_h = nc.dram_tensor("b0r", [128, 9], f32, kind="ExternalInput")
    w1p_h = nc.dram_tensor("w1p", [H2, sum(TASK_DIMS_PAD)], fmm, kind="ExternalInput")
    b1bc_h = nc.dram_tensor("b1bc", [128, sum(TASK_DIMS_PAD)], f32, kind="ExternalInput")
    out_h = [
        nc.dram_tensor(f"out{i}", [int(M[i]), TASK_DIMS[i]], f32, kind="ExternalOutput")
        for i in range(3)
    ]
    eo_h = nc.dram_tensor("eo_scratch", [RT, D], f32)  # internal scratch
    woffs = np.concatenate([[0], np.cumsum(TASK_DIMS_PAD)]).astype(np.int64)

    with tile.TileContext(nc) as tc:
        with (
            tc.tile_pool(name="const", bufs=1) as cp,
            tc.tile_pool(name="psum", bufs=1, space="PSUM") as pp,
        ):
            ident = cp.tile([128, 128], f32, tag="ident")
            make_identity(nc, ident[:])
            sidx = cp.tile([128, 2 * JT], i32, tag="sidx")
            nc.sync.dma_start(out=sidx[:], in_=sidx_h[:, :])
            gts = cp.tile([128, 2 * JT], f32, tag="gts")
            nc.sync.dma_start(out=gts[:], in_=gates_h[:, :])
            b1rt = cp.tile([128, E * DCH], f32, tag="b1rt")
            nc.sync.dma_start(out=b1rt[:], in_=b1r_h[:, :])
            b0rt = cp.tile([128, 9], f32, tag="b0rt")
            nc.sync.dma_start(out=b0rt[:], in_=b0r_h[:, :])

            # ------------- phase 1: expert FFN -------------
            with (
                tc.tile_pool(name="p1w", bufs=2) as wp,
                tc.tile_pool(name="p1x", bufs=2) as xp,
                tc.tile_pool(name="p1h", bufs=2) as hp,
                tc.tile_pool(name="p1eo", bufs=4) as ep,
                tc.tile_pool(name="p1c", bufs=1) as c1p,
            ):
                for e in range(E):
                    b2bct = c1p.tile(
                        [128, D], f32, tag="b2bct", bufs=2, name=f"b2bct_{e}"
                    )
                    nc.scalar.dma_start(
                        out=b2bct[:], in_=b2bc_h[:, e * D : (e + 1) * D]
                    )
                    w1t = []
                    w2t = []
                    xt = []
                    for d in range(DCH):
                        t = wp.tile([128, D], fmm, tag=f"w1_{d}", name=f"w1_{e}_{d}")
                        nc.sync.dma_start(
                            out=t[:], in_=w1_h[e, 128 * d : 128 * (d + 1), :]
                        )
                        w1t.append(t)
                    for d in range(DCH):
                        t = wp.tile([128, D], fmm, tag=f"w2_{d}", name=f"w2_{e}_{d}")
                        nc.scalar.dma_start(
                            out=t[:], in_=w2_h[e, 128 * d : 128 * (d + 1), :]
                        )
                        w2t.append(t)
                    for d in range(DCH):
                        t = xp.tile([128, P_E], fmm, tag=f"x_{d}", name=f"x_{e}_{d}")
                        nc.sync.dma_start(
                            out=t[:],
                            in_=xT_h[
                                128 * d : 128 * (d + 1), e * P_E : (e + 1) * P_E
                            ],
                        )
                        xt.append(t)
                    # L1: hT[h] = relu(sum_d w1[d,h].T @ xT[d] + b1)
                    hT = [
                        hp.tile([128, P_E], fmm, tag=f"hT_{h}", name=f"hT_{e}_{h}")
                        for h in range(DCH)
                    ]
                    for h in range(DCH):
                        for n0, nw in _chunks_bal(P_E, 512):
                            ps = pp.tile(
                                [128, 512], f32, tag="mm", bufs=4, name=f"ps1_{e}_{h}_{n0}"
                            )
                            for d in range(DCH):
                                nc.tensor.matmul(
                                    ps[:, :nw],
                                    lhsT=mmcast(w1t[d][:, 128 * h : 128 * (h + 1)]),
                                    rhs=mmcast(xt[d][:, n0 : n0 + nw]),
                                    start=(d == 0),
                                    stop=(d == DCH - 1),
                                )
                            nc.scalar.activation(
                                hT[h][:, n0 : n0 + nw],
                                ps[:, :nw],
                                AF.Relu,
                                bias=b1rt[:, DCH * e + h : DCH * e + h + 1],
                            )
                    # L2: eo[r-block] = sum_h hT[h][:, r].T @ w2[h] + b2
                    for r0, rw in _chunks(P_E, 128):
                        eos = ep.tile([128, D], f32, tag="eos", name=f"eos_{e}_{r0}")
                        for n0, nw in _chunks(D, 512):
                            ps = pp.tile(
                                [128, 512], f32, tag="mm", bufs=4,
                                name=f"ps2_{e}_{r0}_{n0}",
                            )
                            for h in range(DCH):
                                nc.tensor.matmul(
                                    ps[:rw, :nw],
                                    lhsT=mmcast(hT[h][:, r0 : r0 + rw]),
                                    rhs=mmcast(w2t[h][:, n0 : n0 + nw]),
                                    start=(h == 0),
                                    stop=(h == DCH - 1),
                                )
                            nc.vector.tensor_tensor(
                                out=eos[:rw, n0 : n0 + nw],
                                in0=ps[:rw, :nw],
                                in1=b2bct[:rw, n0 : n0 + nw],
                                op=ALU.add,
                            )
                        nc.scalar.dma_start(
                            out=eo_h[e * P_E + r0 : e * P_E + r0 + rw, :],
                            in_=eos[:rw, :],
                        )

            # ------------- phase 2+3 pools -------------
            with (
                tc.tile_pool(name="p2y", bufs=1) as yp,
                tc.tile_pool(name="p2g", bufs=3) as gp,
                tc.tile_pool(name="p3w", bufs=2) as hwp,
                tc.tile_pool(name="p3c", bufs=1) as c3p,
            ):
                yT = [
                    yp.tile([128, TT], fmm, tag=f"yT{d}", name=f"yT{d}")
                    for d in range(DCH)
                ]
                # ------------- phase 2+3 interleaved -------------
                b1bct = c3p.tile([128, sum(TASK_DIMS_PAD)], f32, tag="b1bct")
                nc.scalar.dma_start(out=b1bct[:], in_=b1bc_h[:, :])
                w1pt = []
                for h in range(H2 // 128):
                    t = c3p.tile([128, sum(TASK_DIMS_PAD)], fmm, tag=f"w1p_{h}")
                    nc.scalar.dma_start(
                        out=t[:], in_=w1p_h[128 * h : 128 * (h + 1), :]
                    )
                    w1pt.append(t)

                def combine_chunk(j):
                    ga = gp.tile([128, D], f32, tag="ga", name=f"ga_{j}")
                    gb = gp.tile([128, D], f32, tag="gb", name=f"gb_{j}")
                    nc.gpsimd.indirect_dma_start(
                        out=ga[:],
                        out_offset=None,
                        in_=eo_h[:, :],
                        in_offset=bass.IndirectOffsetOnAxis(
                            ap=sidx[:, j : j + 1], axis=0
                        ),
                    )
                    nc.gpsimd.indirect_dma_start(
                        out=gb[:],
                        out_offset=None,
                        in_=eo_h[:, :],
                        in_offset=bass.IndirectOffsetOnAxis(
                            ap=sidx[:, JT + j : JT + j + 1], axis=0
                        ),
                    )
                    yj = gp.tile([128, D], f32, tag="yj", name=f"yj_{j}")
                    nc.vector.tensor_scalar_mul(yj[:], ga[:], gts[:, j : j + 1])
                    gu = gp.tile([128, D], f32, tag="gu", name=f"gu_{j}")
                    nc.scalar.activation(
                        gu[:], gb[:], AF.Identity,
                        scale=gts[:, JT + j : JT + j + 1],
                    )
                    nc.vector.tensor_add(yj[:], yj[:], gu[:])
                    for d in range(DCH):
                        tp = pp.tile(
                            [128, 128], f32, tag="tp", bufs=2, name=f"tp_{j}_{d}"
                        )
                        nc.tensor.transpose(
                            tp[:], yj[:, 128 * d : 128 * (d + 1)], ident[:]
                        )
                        if d % 2 == 0:
                            nc.vector.tensor_copy(
                                yT[d][:, 128 * j : 128 * (j + 1)], tp[:]
                            )
                        else:
                            nc.scalar.copy(
                                yT[d][:, 128 * j : 128 * (j + 1)], tp[:]
                            )

                jdone = 0
                for i in range(3):
                    Mi = int(M[i])
                    di = TASK_DIMS[i]
                    jend = (int(offs[i]) + Mi - 1) // 128
                    while jdone <= min(jend, JT - 1):
                        combine_chunk(jdone)
                        jdone += 1
                    w0t = []
                    for d in range(DCH):
                        t = hwp.tile([128, H2], fmm, tag=f"w0_{d}", name=f"w0_{i}_{d}")
                        nc.scalar.dma_start(
                            out=t[:], in_=w0_h[i, 128 * d : 128 * (d + 1), :]
                        )
                        w0t.append(t)
                    hh = [
                        hwp.tile(
                            [128, int(max(M))], fmm, tag=f"hh_{h}", name=f"hh_{i}_{h}"
                        )
                        for h in range(H2 // 128)
                    ]
                    for h in range(H2 // 128):
                        for n0, nw in _chunks_bal(Mi, 512):
                            ps = pp.tile(
                                [128, 512], f32, tag="mm", bufs=4,
                                name=f"ps4_{i}_{h}_{n0}",
                            )
                            for d in range(DCH):
                                nc.tensor.matmul(
                                    ps[:, :nw],
                                    lhsT=mmcast(w0t[d][:, 128 * h : 128 * (h + 1)]),
                                    rhs=mmcast(
                                        yT[d][:, int(offs[i]) + n0 : int(offs[i]) + n0 + nw]
                                    ),
                                    start=(d == 0),
                                    stop=(d == DCH - 1),
                                )
                            # leaky_relu(v) = max(v, NEG_SLOPE * v), v = psum + b0
                            nc.scalar.activation(
                                hh[h][:, n0 : n0 + nw],
                                ps[:, :nw],
                                AF.Identity,
                                bias=b0rt[:, 3 * i + h : 3 * i + h + 1],
                            )
                            lk = gp.tile(
                                [128, 512], fmm, tag="lk", name=f"lk_{i}_{h}_{n0}"
                            )
                            nc.vector.tensor_scalar_mul(
                                lk[:, :nw], hh[h][:, n0 : n0 + nw], NEG_SLOPE
                            )
                            nc.vector.tensor_max(
                                hh[h][:, n0 : n0 + nw],
                                hh[h][:, n0 : n0 + nw],
                                lk[:, :nw],
                            )
                    for r0, rw in _chunks(Mi, 128):
                        for n0, nw in _chunks(TASK_DIMS_PAD[i], 512):
                            wout = min(nw, di - n0)
                            ps = pp.tile(
                                [128, 512], f32, tag="mm", bufs=4,
                                name=f"ps5_{i}_{r0}_{n0}",
                            )
                            for h in range(H2 // 128):
                                nc.tensor.matmul(
                                    ps[:rw, :nw],
                                    lhsT=mmcast(hh[h][:, r0 : r0 + rw]),
                                    rhs=mmcast(
                                        w1pt[h][:, int(woffs[i]) + n0 : int(woffs[i]) + n0 + nw]
                                    ),
                                    start=(h == 0),
                                    stop=(h == H2 // 128 - 1),
                                )
                            osb = gp.tile(
                                [128, 512], f32, tag="osb", name=f"osb_{i}_{r0}_{n0}"
                            )
                            nc.vector.tensor_tensor(
                                out=osb[:rw, :nw],
                                in0=ps[:rw, :nw],
                                in1=b1bct[:rw, int(woffs[i]) + n0 : int(woffs[i]) + n0 + nw],
                                op=ALU.add,
                            )
                            nc.sync.dma_start(
                                out=out_h[i][r0 : r0 + rw, n0 : n0 + wout],
                                in_=osb[:rw, :wout],
                            )
    nc.compile()
    return nc


# ----------------------------------------------------------------------------
# entry point
# ----------------------------------------------------------------------------

def kernel(**inputs):
    global _last_results
    from concourse import bass_utils

    nlp_pooled = np.asarray(inputs["nlp_pooled"], np.float32)
    task_in = np.asarray(inputs["task_in"], np.int32)
    task_emb = np.asarray(inputs["task_emb"], np.float32)
    router_w = np.asarray(inputs["router_w"], np.float32)
    w1 = np.ascontiguousarray(np.asarray(inputs["expert_w1"], np.float32))
    b1 = np.asarray(inputs["expert_b1"], np.float32)
    w2 = np.ascontiguousarray(np.asarray(inputs["expert_w2"], np.float32))
    b2 = np.asarray(inputs["expert_b2"], np.float32)

    meta, per_core = _build_metadata(nlp_pooled, task_in, task_emb, router_w)
    P_E, RT, TT, M, offs = meta["P_E"], meta["RT"], meta["TT"], meta["M"], meta["offs"]
    JT = TT // 128

    # shared (same for all cores) input arrays
    b1r = b1.reshape(E, DCH, 128).transpose(2, 0, 1).reshape(128, E * DCH)
    b1r = np.ascontiguousarray(b1r)
    b2bc = np.ascontiguousarray(np.broadcast_to(b2.reshape(1, E * D), (128, E * D)))
    w0p = np.stack([np.asarray(inputs[f"t{i}_w0"], np.float32) for i in range(3)])
    b0r = np.zeros((128, 9), np.float32)
    for i in range(3):
        b0 = np.asarray(inputs[f"t{i}_b0"], np.float32)
        b0r[:, 3 * i : 3 * i + 3] = b0.reshape(3, 128).T
    TDP = TASK_DIMS_PAD
    toffs = np.concatenate([[0], np.cumsum(TDP)]).astype(np.int64)
    w1p = np.zeros((H2, int(sum(TDP))), np.float32)
    b1p = np.zeros(int(sum(TDP)), np.float32)
    for i in range(3):
        w1p[:, toffs[i] : toffs[i] + TASK_DIMS[i]] = np.asarray(
            inputs[f"t{i}_w1"], np.float32
        )
        b1p[toffs[i] : toffs[i] + TASK_DIMS[i]] = np.asarray(
            inputs[f"t{i}_b1"], np.float32
        )
    b1bc = np.ascontiguousarray(np.broadcast_to(b1p.reshape(1, -1), (128, b1p.size)))

    shared = dict(
        w1=w1, w2=w2, b1r=b1r, b2bc=b2bc,
        w0p=np.ascontiguousarray(w0p), b0r=b0r,
        w1p=np.ascontiguousarray(w1p), b1bc=b1bc,
    )

    in_maps = []
    for c in range(NC):
        pc = per_core[c]
        sidx = np.zeros((128, 2 * JT), np.int32)
        gates = np.zeros((128, 2 * JT), np.float32)
        for k in range(2):
            sidx[:, k * JT : (k + 1) * JT] = (
                pc["slot_row"][k].reshape(JT, 128).T
            )
            gates[:, k * JT : (k + 1) * JT] = pc["gk"][k].reshape(JT, 128).T
        in_maps.append(
            dict(xbufT=pc["xbufT"], sidx=sidx, gates=gates, **shared)
        )

    nc = _build_bass(meta)
    results = bass_utils.run_bass_kernel_spmd(
        nc, in_maps, core_ids=list(range(NC)),
        trace=bool(os.environ.get("BASS_TRACE")),
    )
    _last_results = results

    # assemble outputs
    idx_full = []
    rank = np.zeros(N, np.int64)
    for i in range(3):
        ids = np.nonzero(task_in == i)[0]
        rank[ids] = np.arange(len(ids))
        pad = np.full(N, -1, np.int64)
        pad[: len(ids)] = ids
        idx_full.append(pad.astype(np.int32))

    outs = [np.zeros((N, d), np.float32) for d in TASK_DIMS]
    for c in range(NC):
        pc = per_core[c]
        res = results.results[c]
        for i in range(3):
            ids = pc["placed"][i]
            if len(ids):
                outs[i][rank[ids]] = res[f"out{i}"][: len(ids)]

    return tuple(idx_full), tuple(outs), meta["aux"]


# revision 13
# speedup vs baseline: 1.2187x; 1.0329x over previous
"""MoE routing + task-head kernel for 8 Trainium2 NeuronCores.

Strategy (self-contained, shapes hardcoded from the problem):
  - Host (numpy): router softmax/top-2, capacity positions, gates, aux loss,
    and a balanced token->core assignment (2048 tokens/core) that equalizes
    per-core per-task counts and per-(core,expert) slot counts.  The host
    pre-gathers each core's tokens into an expert-grouped, feature-major
    dispatch buffer xbufT [D, E*P_E].
  - Device (8-way SPMD, one NEFF): per core
      phase 1: per-expert FFN  hT = relu(W1^T-tiles @ xT + b1),
               eo = hT-tiles @ W2 + b2  -> DRAM [E*P_E, D] (row-major)
      phase 2: combine  y = g0*eo[slot0] + g1*eo[slot1] via indirect row
               gathers, then PE-transpose y -> yT [D, TT]
      phase 3: task heads  o_i = lrelu(yT-block @ w0_i + b0_i) @ w1_i + b1_i
  - Host: scatter per-core head outputs back to global row order, return
    (task_indices, task_outs, aux_loss) exactly like the reference.

All matmuls run as float32r (full-rate PE mode, fp32 storage).
"""

import os

import numpy as np

N = 16384
D = 768
E = 8
K = 2
C = 5120
NC = 8
TPC = N // NC
H2 = 384  # task-head hidden dim (D // 2)
NEG_SLOPE = 0.2
TASK_DIMS = (1, 101, 1000)
TASK_DIMS_PAD = (4, 104, 1000)  # padded to mult-of-4 for PE fp32 free dims
DCH = D // 128  # 6 feature chunks
F32R_MM = True  # use float32r matmul mode

_last_results = None  # stashed BassKernelResults (for test harness inspection)


# ----------------------------------------------------------------------------
# host-side routing
# ----------------------------------------------------------------------------

def _softmax(x):
    m = x.max(axis=-1, keepdims=True)
    p = np.exp(x - m)
    return p / p.sum(axis=-1, keepdims=True)


def _route(nlp_pooled, task_in, task_emb, router_w):
    x = nlp_pooled.astype(np.float32)
    logits = (x + task_emb[task_in]) @ router_w
    probs = _softmax(logits)
    order = np.argsort(-probs, axis=-1, kind="stable")
    eidx = order[:, :K].astype(np.int64)
    gate = np.take_along_axis(probs, eidx, axis=-1)
    gate = gate / gate.sum(axis=-1, keepdims=True)

    ef = eidx.reshape(-1)
    pos = np.zeros(N * K, dtype=np.int64)
    for e in range(E):
        sel = ef == e
        pos[sel] = np.arange(sel.sum())
    keep = pos < C
    gflat = (gate.reshape(-1) * keep).reshape(N, K).astype(np.float32)

    oh = np.zeros((N, K, E), np.float32)
    for k in range(K):
        oh[np.arange(N), k, eidx[:, k]] = 1.0
    density = oh.sum(axis=1).mean(axis=0) / K
    aux = np.float32(E * np.sum(density * probs.mean(axis=0)))
    return eidx, gflat, keep, aux


def _assign_tokens(task_in, eidx):
    quota = np.zeros((NC, 3), np.int64)
    for i in range(3):
        cnt = int((task_in == i).sum())
        base, extra = divmod(cnt, NC)
        quota[:, i] = base
        quota[:extra, i] += 1
    ecnt = np.zeros((NC, E), np.int64)
    core_tokens = [[] for _ in range(NC)]
    taken = np.zeros((NC, 3), np.int64)
    for i in range(3):
        ids = np.nonzero(task_in == i)[0]
        for t in ids:
            e0, e1 = eidx[t]
            best, bestscore = -1, None
            for c in range(NC):
                if taken[c, i] >= quota[c, i]:
                    continue
                score = (
                    max(ecnt[c, e0], ecnt[c, e1]),
                    ecnt[c, e0] + ecnt[c, e1],
                    len(core_tokens[c]),
                )
                if bestscore is None or score < bestscore:
                    best, bestscore = c, score
            c = best
            taken[c, i] += 1
            ecnt[c, e0] += 1
            ecnt[c, e1] += 1
            core_tokens[c].append(t)
    # round block sizes up to a multiple of 4 (PE fp32 matmuls need even
    # moving-operand free dims; 4 keeps DMA aligned too)
    M = (quota.max(axis=0) + 3) // 4 * 4
    return [np.asarray(ct, dtype=np.int64) for ct in core_tokens], M, ecnt


def _build_metadata(nlp_pooled, task_in, task_emb, router_w):
    eidx, gate, keep, aux = _route(nlp_pooled, task_in, task_emb, router_w)
    core_tokens, M, ecnt = _assign_tokens(np.asarray(task_in), eidx)

    P_E = (int(ecnt.max()) + 7) // 8 * 8
    RT = E * P_E
    TT = (int(M.sum()) + 127) // 128 * 128
    offs = np.concatenate([[0], np.cumsum(M)]).astype(np.int64)

    x = nlp_pooled.astype(np.float32)
    task_in = np.asarray(task_in)
    per_core = []
    for c in range(NC):
        toks = core_tokens[c]
        tt = task_in[toks]
        lslots = np.full(TT, -1, dtype=np.int64)
        placed = {}
        for i in range(3):
            ids = np.sort(toks[tt == i])
            emax = eidx[ids].max(axis=1) if len(ids) else ids
            ids = ids[np.argsort(emax, kind="stable")]
            lslots[offs[i] : offs[i] + len(ids)] = ids
            placed[i] = ids
        xbufT = np.zeros((D, RT), np.float32)
        slot_row = np.zeros((2, TT), np.int64)
        gk = np.zeros((2, TT), np.float32)
        fill = np.zeros(E, np.int64)
        valid = lslots >= 0
        for ell in np.nonzero(valid)[0]:
            t = lslots[ell]
            for k in range(K):
                if keep[t * K + k]:
                    e = eidx[t, k]
                    r = e * P_E + fill[e]
                    fill[e] += 1
                    xbufT[:, r] = x[t]
                    slot_row[k, ell] = r
                    gk[k, ell] = gate[t, k]
        per_core.append(dict(xbufT=xbufT, slot_row=slot_row, gk=gk, placed=placed))
    JT = TT // 128
    elims = np.zeros(JT, np.int64)
    for pc in per_core:
        sr = pc["slot_row"].reshape(2, JT, 128)
        elims = np.maximum(elims, sr.max(axis=(0, 2)) // P_E + 1)
    meta = dict(P_E=P_E, RT=RT, TT=TT, M=M, offs=offs, aux=aux,
                elims=[int(x) for x in elims])
    return meta, per_core


# ----------------------------------------------------------------------------
# device kernel
# ----------------------------------------------------------------------------

def _chunks(total, step):
    out, n0 = [], 0
    while n0 < total:
        out.append((n0, min(step, total - n0)))
        n0 += step
    return out


def _chunks_bal(total, maxstep):
    """Near-equal chunks (each a multiple of 4, <= maxstep) so no chunk is a
    tiny ragged tail whose matmul can't hide its LDWEIGHTS."""
    nch = -(-total // maxstep)
    step = (-(-total // nch) + 3) // 4 * 4
    return _chunks(total, step)


def _build_bass(meta):
    import concourse.bacc as bacc
    import concourse.bass as bass
    import concourse.mybir as mybir
    import concourse.tile as tile
    from concourse.masks import make_identity

    f32 = mybir.dt.float32
    f32r = mybir.dt.float32r
    i32 = mybir.dt.int32
    AF = mybir.ActivationFunctionType
    ALU = mybir.AluOpType

    fmm = f32r if F32R_MM else f32

    def mmcast(ap):
        return ap

    P_E, RT, TT, M, offs = meta["P_E"], meta["RT"], meta["TT"], meta["M"], meta["offs"]
    JT = TT // 128  # combine chunks

    nc = bacc.Bacc("TRN2", target_bir_lowering=False, debug=False)

    # --- external IO ------------------------------------------------------
    xT_h = nc.dram_tensor("xbufT", [D, RT], fmm, kind="ExternalInput")
    sidx_h = nc.dram_tensor("sidx", [128, 2 * JT], i32, kind="ExternalInput")
    gates_h = nc.dram_tensor("gates", [128, 2 * JT], f32, kind="ExternalInput")
    w1_h = nc.dram_tensor("w1", [E, D, D], fmm, kind="ExternalInput")
    w2_h = nc.dram_tensor("w2", [E, D, D], fmm, kind="ExternalInput")
    b1r_h = nc.dram_tensor("b1r", [128, E * DCH], f32, kind="ExternalInput")
    b2bc_h = nc.dram_tensor("b2bc", [128, E * D], f32, kind="ExternalInput")
    w0_h = nc.dram_tensor("w0p", [3, D, H2], fmm, kind="ExternalInput")
    b0r_h = nc.dram_tensor("b0r", [128, 9], f32, kind="ExternalInput")
    w1p_h = nc.dram_tensor("w1p", [H2, sum(TASK_DIMS_PAD)], fmm, kind="ExternalInput")
    b1bc_h = nc.dram_tensor("b1bc", [128, sum(TASK_DIMS_PAD)], f32, kind="ExternalInput")
    out_h = [
        nc.dram_tensor(f"out{i}", [int(M[i]), TASK_DIMS[i]], f32, kind="ExternalOutput")
        for i in range(3)
    ]
    eo_h = nc.dram_tensor("eo_scratch", [RT, D], f32)  # internal scratch
    woffs = np.concatenate([[0], np.cumsum(TASK_DIMS_PAD)]).astype(np.int64)

    with tile.TileContext(nc) as tc:
        with (
            tc.tile_pool(name="const", bufs=1) as cp,
            tc.tile_pool(name="psum", bufs=1, space="PSUM") as pp,
        ):
            ident = cp.tile([128, 128], f32, tag="ident")
            make_identity(nc, ident[:])
            sidx = cp.tile([128, 2 * JT], i32, tag="sidx")
            nc.sync.dma_start(out=sidx[:], in_=sidx_h[:, :])
            gts = cp.tile([128, 2 * JT], f32, tag="gts")
            nc.sync.dma_start(out=gts[:], in_=gates_h[:, :])
            b1rt = cp.tile([128, E * DCH], f32, tag="b1rt")
            nc.sync.dma_start(out=b1rt[:], in_=b1r_h[:, :])
            b0rt = cp.tile([128, 9], f32, tag="b0rt")
            nc.sync.dma_start(out=b0rt[:], in_=b0r_h[:, :])

            # ------------- phase 1: expert FFN -------------
            with (
                tc.tile_pool(name="p1w", bufs=2) as wp,
                tc.tile_pool(name="p1x", bufs=2) as xp,
                tc.tile_pool(name="p1h", bufs=2) as hp,
                tc.tile_pool(name="p1eo", bufs=4) as ep,
                tc.tile_pool(name="p1c", bufs=1) as c1p,
            ):
                for e in range(E):
                    b2bct = c1p.tile(
                        [128, D], f32, tag="b2bct", bufs=2, name=f"b2bct_{e}"
                    )
                    nc.scalar.dma_start(
                        out=b2bct[:], in_=b2bc_h[:, e * D : (e + 1) * D]
                    )
                    w1t = []
                    w2t = []
                    xt = []
                    for d in range(DCH):
                        t = wp.tile([128, D], fmm, tag=f"w1_{d}", name=f"w1_{e}_{d}")
                        nc.sync.dma_start(
                            out=t[:], in_=w1_h[e, 128 * d : 128 * (d + 1), :]
                        )
                        w1t.append(t)
                    for d in range(DCH):
                        t = wp.tile([128, D], fmm, tag=f"w2_{d}", name=f"w2_{e}_{d}")
                        nc.scalar.dma_start(
                            out=t[:], in_=w2_h[e, 128 * d : 128 * (d + 1), :]
                        )
                        w2t.append(t)
                    for d in range(DCH):
                        t = xp.tile([128, P_E], fmm, tag=f"x_{d}", name=f"x_{e}_{d}")
                        nc.sync.dma_start(
                            out=t[:],
                            in_=xT_h[
                                128 * d : 128 * (d + 1), e * P_E : (e + 1) * P_E
                            ],
                        )
                        xt.append(t)
                    # L1: hT[h] = relu(sum_d w1[d,h].T @ xT[d] + b1)
                    hT = [
                        hp.tile([128, P_E], fmm, tag=f"hT_{h}", name=f"hT_{e}_{h}")
                        for h in range(DCH)
                    ]
                    for h in range(DCH):
                        for n0, nw in _chunks_bal(P_E, 512):
                            ps = pp.tile(
                                [128, 512], f32, tag="mm", bufs=4, name=f"ps1_{e}_{h}_{n0}"
                            )
                            for d in range(DCH):
                                nc.tensor.matmul(
                                    ps[:, :nw],
                                    lhsT=mmcast(w1t[d][:, 128 * h : 128 * (h + 1)]),
                                    rhs=mmcast(xt[d][:, n0 : n0 + nw]),
                                    start=(d == 0),
                                    stop=(d == DCH - 1),
                                )
                            nc.scalar.activation(
                                hT[h][:, n0 : n0 + nw],
                                ps[:, :nw],
                                AF.Relu,
                                bias=b1rt[:, DCH * e + h : DCH * e + h + 1],
                            )
                    # L2: eo[r-block] = sum_h hT[h][:, r].T @ w2[h] + b2
                    for r0, rw in _chunks(P_E, 128):
                        eos = ep.tile([128, D], f32, tag="eos", name=f"eos_{e}_{r0}")
                        for n0, nw in _chunks(D, 512):
                            ps = pp.tile(
                                [128, 512], f32, tag="mm", bufs=4,
                                name=f"ps2_{e}_{r0}_{n0}",
                            )
                            for h in range(DCH):
                                nc.tensor.matmul(
                                    ps[:rw, :nw],
                                    lhsT=mmcast(hT[h][:, r0 : r0 + rw]),
                                    rhs=mmcast(w2t[h][:, n0 : n0 + nw]),
                                    start=(h == 0),
                                    stop=(h == DCH - 1),
                                )
                            nc.vector.tensor_tensor(
                                out=eos[:rw, n0 : n0 + nw],
                                in0=ps[:rw, :nw],
                                in1=b2bct[:rw, n0 : n0 + nw],
                                op=ALU.add,
                            )
                        nc.scalar.dma_start(
                            out=eo_h[e * P_E + r0 : e * P_E + r0 + rw, :],
                            in_=eos[:rw, :],
                        )

            # ------------- phase 2+3 pools -------------
            with (
                tc.tile_pool(name="p2y", bufs=1) as yp,
                tc.tile_pool(name="p2g", bufs=3) as gp,
                tc.tile_pool(name="p3w", bufs=2) as hwp,
                tc.tile_pool(name="p3c", bufs=1) as c3p,
            ):
                yT = [
                    yp.tile([128, TT], fmm, tag=f"yT{d}", name=f"yT{d}")
                    for d in range(DCH)
                ]
                # ------------- phase 2+3 interleaved -------------
                b1bct = c3p.tile([128, sum(TASK_DIMS_PAD)], f32, tag="b1bct")
                nc.scalar.dma_start(out=b1bct[:], in_=b1bc_h[:, :])
                w1pt = []
                for h in range(H2 // 128):
                    t = c3p.tile([128, sum(TASK_DIMS_PAD)], fmm, tag=f"w1p_{h}")
                    nc.scalar.dma_start(
                        out=t[:], in_=w1p_h[128 * h : 128 * (h + 1), :]
                    )
                    w1pt.append(t)

                def combine_chunk(j):
                    rl = meta["elims"][j] * P_E
                    ga = gp.tile([128, D], f32, tag="ga", name=f"ga_{j}")
                    gb = gp.tile([128, D], f32, tag="gb", name=f"gb_{j}")
                    nc.gpsimd.indirect_dma_start(
                        out=ga[:],
                        out_offset=None,
                        in_=eo_h[0:rl, :],
                        in_offset=bass.IndirectOffsetOnAxis(
                            ap=sidx[:, j : j + 1], axis=0
                        ),
                    )
                    nc.gpsimd.indirect_dma_start(
                        out=gb[:],
                        out_offset=None,
                        in_=eo_h[0:rl, :],
                        in_offset=bass.IndirectOffsetOnAxis(
                            ap=sidx[:, JT + j : JT + j + 1], axis=0
                        ),
                    )
                    yj = gp.tile([128, D], f32, tag="yj", name=f"yj_{j}")
                    nc.vector.tensor_scalar_mul(yj[:], ga[:], gts[:, j : j + 1])
                    gu = gp.tile([128, D], f32, tag="gu", name=f"gu_{j}")
                    nc.scalar.activation(
                        gu[:], gb[:], AF.Identity,
                        scale=gts[:, JT + j : JT + j + 1],
                    )
                    nc.vector.tensor_add(yj[:], yj[:], gu[:])
                    for d in range(DCH):
                        tp = pp.tile(
                            [128, 128], f32, tag="tp", bufs=2, name=f"tp_{j}_{d}"
                        )
                        nc.tensor.transpose(
                            tp[:], yj[:, 128 * d : 128 * (d + 1)], ident[:]
                        )
                        if d % 2 == 0:
                            nc.vector.tensor_copy(
                                yT[d][:, 128 * j : 128 * (j + 1)], tp[:]
                            )
                        else:
                            nc.scalar.copy(
                                yT[d][:, 128 * j : 128 * (j + 1)], tp[:]
                            )

                jdone = 0
                for i in range(3):
                    Mi = int(M[i])
                    di = TASK_DIMS[i]
                    jend = (int(offs[i]) + Mi - 1) // 128
                    while jdone <= min(jend, JT - 1):
                        combine_chunk(jdone)
                        jdone += 1
                    w0t = []
                    for d in range(DCH):
                        t = hwp.tile([128, H2], fmm, tag=f"w0_{d}", name=f"w0_{i}_{d}")
                        nc.scalar.dma_start(
                            out=t[:], in_=w0_h[i, 128 * d : 128 * (d + 1), :]
                        )
                        w0t.append(t)
                    hh = [
                        hwp.tile(
                            [128, int(max(M))], fmm, tag=f"hh_{h}", name=f"hh_{i}_{h}"
                        )
                        for h in range(H2 // 128)
                    ]
                    for h in range(H2 // 128):
                        for n0, nw in _chunks_bal(Mi, 512):
                            ps = pp.tile(
                                [128, 512], f32, tag="mm", bufs=4,
                                name=f"ps4_{i}_{h}_{n0}",
                            )
                            for d in range(DCH):
                                nc.tensor.matmul(
                                    ps[:, :nw],
                                    lhsT=mmcast(w0t[d][:, 128 * h : 128 * (h + 1)]),
                                    rhs=mmcast(
                                        yT[d][:, int(offs[i]) + n0 : int(offs[i]) + n0 + nw]
                                    ),
                                    start=(d == 0),
                                    stop=(d == DCH - 1),
                                )
                            # leaky_relu(v) = max(v, NEG_SLOPE * v), v = psum + b0
                            nc.scalar.activation(
                                hh[h][:, n0 : n0 + nw],
                                ps[:, :nw],
                                AF.Identity,
                                bias=b0rt[:, 3 * i + h : 3 * i + h + 1],
                            )
                            lk = gp.tile(
                                [128, 512], fmm, tag="lk", name=f"lk_{i}_{h}_{n0}"
                            )
                            nc.vector.tensor_scalar_mul(
                                lk[:, :nw], hh[h][:, n0 : n0 + nw], NEG_SLOPE
                            )
                            nc.vector.tensor_max(
                                hh[h][:, n0 : n0 + nw],
                                hh[h][:, n0 : n0 + nw],
                                lk[:, :nw],
                            )
                    for r0, rw in _chunks(Mi, 128):
                        for n0, nw in _chunks(TASK_DIMS_PAD[i], 512):
                            wout = min(nw, di - n0)
                            ps = pp.tile(
                                [128, 512], f32, tag="mm", bufs=4,
                                name=f"ps5_{i}_{r0}_{n0}",
                            )
                            for h in range(H2 // 128):
                                nc.tensor.matmul(
                                    ps[:rw, :nw],
                                    lhsT=mmcast(hh[h][:, r0 : r0 + rw]),
                                    rhs=mmcast(
                                        w1pt[h][:, int(woffs[i]) + n0 : int(woffs[i]) + n0 + nw]
                                    ),
                                    start=(h == 0),
                                    stop=(h == H2 // 128 - 1),
                                )
                            osb = gp.tile(
                                [128, 512], f32, tag="osb", name=f"osb_{i}_{r0}_{n0}"
                            )
                            nc.vector.tensor_tensor(
                                out=osb[:rw, :nw],
                                in0=ps[:rw, :nw],
                                in1=b1bct[:rw, int(woffs[i]) + n0 : int(woffs[i]) + n0 + nw],
                                op=ALU.add,
                            )
                            nc.sync.dma_start(
                                out=out_h[i][r0 : r0 + rw, n0 : n0 + wout],
                                in_=osb[:rw, :wout],
                            )
    nc.compile()
    return nc


# ----------------------------------------------------------------------------
# entry point
# ----------------------------------------------------------------------------

def kernel(**inputs):
    global _last_results
    from concourse import bass_utils

    nlp_pooled = np.asarray(inputs["nlp_pooled"], np.float32)
    task_in = np.asarray(inputs["task_in"], np.int32)
    task_emb = np.asarray(inputs["task_emb"], np.float32)
    router_w = np.asarray(inputs["router_w"], np.float32)
    w1 = np.ascontiguousarray(np.asarray(inputs["expert_w1"], np.float32))
    b1 = np.asarray(inputs["expert_b1"], np.float32)
    w2 = np.ascontiguousarray(np.asarray(inputs["expert_w2"], np.float32))
    b2 = np.asarray(inputs["expert_b2"], np.float32)

    meta, per_core = _build_metadata(nlp_pooled, task_in, task_emb, router_w)
    P_E, RT, TT, M, offs = meta["P_E"], meta["RT"], meta["TT"], meta["M"], meta["offs"]
    JT = TT // 128

    # shared (same for all cores) input arrays
    b1r = b1.reshape(E, DCH, 128).transpose(2, 0, 1).reshape(128, E * DCH)
    b1r = np.ascontiguousarray(b1r)
    b2bc = np.ascontiguousarray(np.broadcast_to(b2.reshape(1, E * D), (128, E * D)))
    w0p = np.stack([np.asarray(inputs[f"t{i}_w0"], np.float32) for i in range(3)])
    b0r = np.zeros((128, 9), np.float32)
    for i in range(3):
        b0 = np.asarray(inputs[f"t{i}_b0"], np.float32)
        b0r[:, 3 * i : 3 * i + 3] = b0.reshape(3, 128).T
    TDP = TASK_DIMS_PAD
    toffs = np.concatenate([[0], np.cumsum(TDP)]).astype(np.int64)
    w1p = np.zeros((H2, int(sum(TDP))), np.float32)
    b1p = np.zeros(int(sum(TDP)), np.float32)
    for i in range(3):
        w1p[:, toffs[i] : toffs[i] + TASK_DIMS[i]] = np.asarray(
            inputs[f"t{i}_w1"], np.float32
        )
        b1p[toffs[i] : toffs[i] + TASK_DIMS[i]] = np.asarray(
            inputs[f"t{i}_b1"], np.float32
        )
    b1bc = np.ascontiguousarray(np.broadcast_to(b1p.reshape(1, -1), (128, b1p.size)))

    shared = dict(
        w1=w1, w2=w2, b1r=b1r, b2bc=b2bc,
        w0p=np.ascontiguousarray(w0p), b0r=b0r,
        w1p=np.ascontiguousarray(w1p), b1bc=b1bc,
    )

    in_maps = []
    for c in range(NC):
        pc = per_core[c]
        sidx = np.zeros((128, 2 * JT), np.int32)
        gates = np.zeros((128, 2 * JT), np.float32)
        for k in range(2):
            sidx[:, k * JT : (k + 1) * JT] = (
                pc["slot_row"][k].reshape(JT, 128).T
            )
            gates[:, k * JT : (k + 1) * JT] = pc["gk"][k].reshape(JT, 128).T
        in_maps.append(
            dict(xbufT=pc["xbufT"], sidx=sidx, gates=gates, **shared)
        )

    nc = _build_bass(meta)
    results = bass_utils.run_bass_kernel_spmd(
        nc, in_maps, core_ids=list(range(NC)),
        trace=bool(os.environ.get("BASS_TRACE")),
    )
    _last_results = results

    # assemble outputs
    idx_full = []
    rank = np.zeros(N, np.int64)
    for i in range(3):
        ids = np.nonzero(task_in == i)[0]
        rank[ids] = np.arange(len(ids))
        pad = np.full(N, -1, np.int64)
        pad[: len(ids)] = ids
        idx_full.append(pad.astype(np.int32))

    outs = [np.zeros((N, d), np.float32) for d in TASK_DIMS]
    for c in range(NC):
        pc = per_core[c]
        res = results.results[c]
        for i in range(3):
            ids = pc["placed"][i]
            if len(ids):
                outs[i][rank[ids]] = res[f"out{i}"][: len(ids)]

    return tuple(idx_full), tuple(outs), meta["aux"]
